# revision 3
# baseline (speedup 1.0000x reference)
"""Cross-attention block (LN -> QKV -> full softmax attention -> proj + residual)
as a Bass/Tile kernel for 8 Trainium2 NeuronCores — fp8 DoubleRow variant.

Sharding (hardcoded for B=4, H=W=64, C=U=256):
  core c handles batch b = c//2 and query-half h = c%2 (2048 of 4096 query
  positions), with K/V computed from the full 4096-position context of batch b
  (replicated inside the 2-core group). No collectives needed.

Key differences vs the f32r baseline (235us):
  - All big matmuls run fp8e4 DoubleRow (contraction 256 per pass): QKV over
    the two C-halves, scores over the two U-halves, attention over key-tile
    pairs. Measured 220ns per [K=256]x[128,512] pass vs 502ns for the f32r
    pair.
  - Host folds gamma into Wq (Wq' = diag(gamma) Wq, bq' = beta@Wq + bq) so
    the q path uses raw (pre-gamma) layernorm output; gamma/beta (+bp) are
    applied to the residual on the DVE during the attention phase instead of
    the startup critical path.
  - exp outputs fp8 into a persistent p-slab ([keys, keytile, blk, q]); bias
    -EXP_SHIFT keeps exp in fp8e4 range (cancels in the normalization).
    One exp instruction covers a 2-bank psum pair (the two key tiles).
  - softmax denominator: ones-lhsT fp8 DoubleRow matmuls over the p-slab on
    the PE (218ns each), not DVE adds.
  - epilogue: one fused DVE scalar_tensor_tensor (proj * 1/denom + residual).
  - PSUM: 4 banks po + 4 banks (2x2-bank) score tiles = 8.
"""

import numpy as np

P = 128
C = 256
U = 256
NQ = 2048          # queries per core
NK = 4096          # keys per core
QT = NQ // P       # 16 query tiles
KT = NK // P       # 32 key tiles
IB = 512           # psum block width (queries)
SB = 1024          # superblock: 1024 queries
NSB = NQ // SB     # 2
NPAIR = KT // 2    # 16 key-tile pairs
SCALE = float(U) ** -0.5
EXP_SHIFT = 4.0    # exp(score*SCALE - SHIFT): keeps p <= e^{smax-4} << 240
LN_EPS = 1e-3

_CACHE = {}
LAST_RESULTS = None


def _build_bass():
    import concourse.bass as bass
    import concourse.tile as tile
    from concourse import bacc, mybir
    from concourse.masks import make_identity
    from concourse.alu_op_type import AluOpType

    f32 = mybir.dt.float32
    f32r = mybir.dt.float32r
    fp8 = mybir.dt.float8e4
    AF = mybir.ActivationFunctionType
    PM = mybir.MatmulPerfMode

    nc = bacc.Bacc("TRN2", debug=False, num_devices=8)

    x_d = nc.dram_tensor("x", [NQ, C], f32, kind="ExternalInput").ap()
    ctxT_d = nc.dram_tensor("ctxT", [C, NK], f32, kind="ExternalInput").ap()
    w_d = {
        name: nc.dram_tensor(name, [C, U], f32, kind="ExternalInput").ap()
        for name in ("Wq", "Wk", "Wv", "Wp")
    }
    b_d = {
        name: nc.dram_tensor(name, [U], f32, kind="ExternalInput").ap()
        for name in ("bq", "bk", "bv")
    }
    gamma_d = nc.dram_tensor("gamma", [C], f32, kind="ExternalInput").ap()
    beta_d = nc.dram_tensor("beta", [C], f32, kind="ExternalInput").ap()
    out_d = nc.dram_tensor("out", [NQ, C], f32, kind="ExternalOutput").ap()

    def bcast(ap1d):
        # [N] dram vector -> [P, N] broadcast read (partition step 0)
        return bass.AP(tensor=ap1d.tensor, offset=ap1d.offset, ap=[[0, P], *ap1d.ap])

    with tile.TileContext(nc) as tc:
        from contextlib import ExitStack

        with ExitStack() as es:
            singles = es.enter_context(tc.tile_pool(name="singles", bufs=1))
            psum = es.enter_context(tc.tile_pool(name="psum", bufs=2, space="PSUM"))
            work = es.enter_context(tc.tile_pool(name="work", bufs=4))
            ln = es.enter_context(tc.tile_pool(name="ln", bufs=4))
            inv_pool = es.enter_context(tc.tile_pool(name="inv_pool", bufs=4))
            fin_pool = es.enter_context(tc.tile_pool(name="fin_pool", bufs=4))

            # ---- constants ----
            ident = singles.tile([P, P], f32)
            make_identity(nc, ident)
            eps_t = singles.tile([P, 1], f32)
            nc.vector.memset(eps_t, LN_EPS)
            one11 = singles.tile([1, 1], f32)
            nc.vector.memset(one11, 1.0)
            ones_f = singles.tile([P, 2, 16], f32)
            nc.vector.memset(ones_f, 1.0)
            ones8 = singles.tile([P, 2, 16], fp8)
            nc.vector.tensor_copy(out=ones8, in_=ones_f)
            shift_t = singles.tile([P, 1], f32)
            nc.vector.memset(shift_t, -EXP_SHIFT)

            # DMA order on the sync queue is the startup critical path:
            # Wk + biases first (kT needs them), then ctxT chunks interleaved
            # with the x tiles (LN runs on DVE/ACT while PE does kT/v).
            w_stage = {}
            w_sb = {}

            def dma_w(name, eng=None):
                t0 = work.tile([P, 2, U], f32, tag="wstage", name=f"sb0_{name}", bufs=2)
                (eng or nc.sync).dma_start(out=t0, in_=w_d[name].rearrange("(a p) u -> p a u", p=P))
                w_stage[name] = t0

            def cast_w(name, dt):
                t = singles.tile([P, 2, U], dt, name=f"sb_{name}")
                nc.vector.tensor_copy(out=t, in_=w_stage[name])
                w_sb[name] = t

            dma_w("Wk", nc.gpsimd)
            dma_w("Wv", nc.gpsimd)
            bk_t = singles.tile([P, 2], f32)
            nc.gpsimd.dma_start(out=bk_t, in_=b_d["bk"].rearrange("(a p) -> p a", p=P))
            bq_t = singles.tile([P, 2], f32)
            nc.gpsimd.dma_start(out=bq_t, in_=b_d["bq"].rearrange("(a p) -> p a", p=P))
            cast_w("Wk", fp8)
            cast_w("Wv", fp8)
            bv_b = singles.tile([P, C], f32)
            nc.gpsimd.dma_start(out=bv_b, in_=bcast(b_d["bv"]))
            gamma_b = singles.tile([P, C], f32)
            nc.gpsimd.dma_start(out=gamma_b, in_=bcast(gamma_d))
            beta_b = singles.tile([P, C], f32)
            nc.gpsimd.dma_start(out=beta_b, in_=bcast(beta_d))

            # ---- persistent slabs ----
            xn = singles.tile([P, QT, C], f32)         # x_n raw (pre gamma/beta)
            xnT8 = singles.tile([P, 2, NQ], fp8)       # x_n raw transposed, fp8
            kT = singles.tile([P, 2, NK], fp8)         # k transposed [U, keys]
            qT = singles.tile([P, 2, NQ], fp8)         # q transposed [U, queries]
            v_sb = singles.tile([P, KT, C], fp8)       # v natural [keys(P), C]
            atT = singles.tile([P, 2, NQ], f32r)       # attn-out unnormalized [C, q]
            # p slab: exp(scores) fp8, [keys(P), keytile, blk, q] per superblock
            p_slab = singles.tile([P, KT, 2, IB], fp8)
            xnf = singles.tile([P, QT, C], f32)        # residual: xn*gamma + beta(+bp)

            # ctxT arrives pre-transposed from the host; chunked DMA with the
            # x tiles interleaved; each chunk is cast to fp8 on the DVE for
            # the DoubleRow kT/v matmuls. ctx pool released after the v loop.
            ctxp = tc.alloc_tile_pool(name="ctxp", bufs=1)
            ctxT = ctxp.tile([P, 2, NK], f32)       # context transposed [C, keys]
            ctxT8 = singles.tile([P, 2, NK], fp8)
            ctxT_src = ctxT_d.rearrange("(a p) j -> p a j", p=P)
            x_t3 = x_d.rearrange("(t p) c -> t p c", p=P)
            x_tiles = []
            NCH = 8
            CHW = NK // NCH
            for ch in range(NCH):
                nc.sync.dma_start(
                    out=ctxT[:, :, ch * CHW:(ch + 1) * CHW],
                    in_=ctxT_src[:, :, ch * CHW:(ch + 1) * CHW],
                )
                nc.vector.tensor_copy(
                    out=ctxT8[:, :, ch * CHW:(ch + 1) * CHW],
                    in_=ctxT[:, :, ch * CHW:(ch + 1) * CHW],
                )
                for t in range(ch * 2, ch * 2 + 2):
                    x_t = work.tile([P, C], f32, tag="x", name=f"x_{t}", bufs=8)
                    nc.gpsimd.dma_start(out=x_t, in_=x_t3[t])
                    x_tiles.append(x_t)

            # ---- kT[u, j] = sum_c Wk[c, u] * ctx[j, c] (first PE work) ----
            for n in range(NK // 512):
                for b2 in range(2):
                    ps = psum.tile([P, 512], f32, tag="po", bufs=4, name="ps_k")
                    nc.tensor.matmul(
                        ps,
                        lhsT=w_sb["Wk"][:, :, b2 * P:(b2 + 1) * P],
                        rhs=ctxT8[:, :, n * 512:(n + 1) * 512],
                        start=True,
                        stop=True,
                        perf_mode=PM.DoubleRow,
                    )
                    nc.scalar.activation(
                        out=kT[:, b2, n * 512:(n + 1) * 512],
                        in_=ps,
                        func=AF.Identity,
                        bias=bk_t[:, b2:b2 + 1],
                    )

            dma_w("Wq")
            dma_w("Wp")

            def emit_ln(t):
                # layernorm (no gamma/beta: folded into Wq / applied later)
                x_t = x_tiles[t]
                st = ln.tile([P, 6], f32, tag="st")
                nc.vector.bn_stats(out=st, in_=x_t)
                mv = ln.tile([P, 2], f32, tag="mv")
                nc.vector.bn_aggr(out=mv, in_=st)
                rstd = ln.tile([P, 1], f32, tag="rstd")
                nc.scalar.activation(out=rstd, in_=mv[:, 1:2], func=AF.Sqrt, bias=eps_t)
                nc.vector.reciprocal(rstd, rstd)
                nmr = ln.tile([P, 1], f32, tag="nmr")
                nc.vector.tensor_mul(nmr, mv[:, 0:1], rstd)
                nc.vector.tensor_scalar_mul(nmr, nmr, -1.0)
                # x_n = x * rstd - mu * rstd
                nc.scalar.activation(
                    out=xn[:, t, :], in_=x_t, func=AF.Identity, bias=nmr, scale=rstd
                )

            # ---- v natural [j, c] fp8, LN interleaved ----
            for t in range(KT):
                ps = psum.tile([P, C], f32, tag="po", bufs=4, name="ps_v")
                nc.tensor.matmul(
                    ps,
                    lhsT=ctxT8[:, :, t * P:(t + 1) * P],
                    rhs=w_sb["Wv"],
                    start=True,
                    stop=True,
                    perf_mode=PM.DoubleRow,
                )
                nc.vector.tensor_add(v_sb[:, t, :], ps, bv_b)
                if t < QT:
                    emit_ln(t)
                if t == 20:
                    cast_w("Wq", fp8)
                    cast_w("Wp", f32r)

            ctxp.release()

            # ---- transpose raw x_n (PE), write fp8 on the ACT copy-out ----
            for t in range(QT):
                for a in range(2):
                    pt = psum.tile([P, P], f32, tag="po", bufs=4, name="pt_xn")
                    nc.tensor.transpose(pt, xn[:, t, a * P:(a + 1) * P], ident)
                    nc.scalar.copy(out=xnT8[:, a, t * P:(t + 1) * P], in_=pt)

            # ---- qT[u, i] = sum_c Wq'[c, u] * xn_raw[i, c] ----
            for n in range(NQ // 512):
                for b2 in range(2):
                    ps = psum.tile([P, 512], f32, tag="po", bufs=4, name="ps_q")
                    nc.tensor.matmul(
                        ps,
                        lhsT=w_sb["Wq"][:, :, b2 * P:(b2 + 1) * P],
                        rhs=xnT8[:, :, n * 512:(n + 1) * 512],
                        start=True,
                        stop=True,
                        perf_mode=PM.DoubleRow,
                    )
                    nc.scalar.activation(
                        out=qT[:, b2, n * 512:(n + 1) * 512],
                        in_=ps,
                        func=AF.Identity,
                        bias=bq_t[:, b2:b2 + 1],
                    )

            # ---- attention superblocks ----
            def run_superblock(sb, tasks=None):
                po = [
                    [
                        psum.tile([P, IB], f32, tag="po", bufs=4,
                                  name=f"po{sb}_{blk}_{ci}")
                        for ci in range(2)
                    ]
                    for blk in range(2)
                ]

                def emit_attn(t):
                    for ci in range(2):
                        for blk in range(2):
                            nc.tensor.matmul(
                                po[blk][ci],
                                lhsT=v_sb[:, 2 * t:2 * t + 2, ci * P:(ci + 1) * P],
                                rhs=p_slab[:, 2 * t:2 * t + 2, blk, :],
                                start=(t == 0),
                                stop=(t == NPAIR - 1),
                                perf_mode=PM.DoubleRow,
                            )

                pend = None
                for t in range(NPAIR):
                    for blk in range(2):
                        ps2 = psum.tile([P, 2, IB], f32, tag="s", bufs=2,
                                        name="ps_s")
                        for m in range(2):
                            j = 2 * t + m
                            nc.tensor.matmul(
                                ps2[:, m, :],
                                lhsT=kT[:, :, j * P:(j + 1) * P],
                                rhs=qT[:, :, sb * SB + blk * IB:sb * SB + (blk + 1) * IB],
                                start=True,
                                stop=True,
                                perf_mode=PM.DoubleRow,
                            )
                        nc.scalar.activation(
                            out=p_slab[:, 2 * t:2 * t + 2, blk, :],
                            in_=ps2,
                            func=AF.Exp,
                            scale=SCALE,
                            bias=shift_t,
                        )
                    # deferred residual prep on the otherwise-idle DVE:
                    # xnf = xn_raw * gamma + (beta + bp)
                    if sb == 0 and t < QT:
                        nc.vector.tensor_mul(xnf[:, t, :], xn[:, t, :], gamma_b)
                        nc.vector.tensor_add(xnf[:, t, :], xnf[:, t, :], beta_b)
                    # prior superblock's proj/inv epilogue rides mid-loop
                    if tasks is not None and t in tasks:
                        tasks[t]()
                    # 1-deep software pipeline for the attention matmuls
                    if pend is not None:
                        emit_attn(pend)
                    pend = t
                emit_attn(pend)
                return po

            def early_epilogue(sb, po):
                # free the po psum banks (DVE copies), then denominator
                # matmuls (PE) into a tag-s psum slot.
                for blk in range(2):
                    qlo = sb * SB + blk * IB
                    for ci in range(2):
                        nc.vector.tensor_copy(
                            out=atT[:, ci, qlo:qlo + IB], in_=po[blk][ci]
                        )
                invs = []
                for blk in range(2):
                    ps_d = psum.tile([2, IB], f32, tag="s", name="ps_d")
                    for t in range(NPAIR):
                        nc.tensor.matmul(
                            ps_d[0:1, :],
                            lhsT=ones8[:, :, 0:1],
                            rhs=p_slab[:, 2 * t:2 * t + 2, blk, :],
                            start=(t == 0),
                            stop=(t == NPAIR - 1),
                            perf_mode=PM.DoubleRow,
                        )
                    inv_row = inv_pool.tile([1, IB], f32, tag="invrow")
                    nc.vector.reciprocal(inv_row, ps_d[0:1, :])
                    invs.append(inv_row)
                return invs

            def late_tasks(sb, invs):
                # chunked: slot 8 = all 1/denom transposes; slots 10..13 =
                # two proj tiles each (f32r + fused normalize/residual DVE).
                inv_ts = []
                tasks = {}

                def t_inv():
                  for blk in range(2):
                    for s in range(IB // P):
                        ps_i = psum.tile([P, 1], f32, tag="s", name="ps_i")
                        nc.tensor.matmul(
                            ps_i,
                            lhsT=invs[blk][0:1, s * P:(s + 1) * P],
                            rhs=one11,
                            start=True,
                            stop=True,
                        )
                        inv_t = inv_pool.tile([P, 1], f32, tag="invt", bufs=8)
                        nc.vector.tensor_copy(out=inv_t, in_=ps_i)
                        inv_ts.append(inv_t)

                tasks[8] = t_inv

                def mk_proj(pair):
                  def t_proj():
                    for blk, s in pair:
                        t = (sb * SB + blk * IB) // P + s
                        ps_p = psum.tile([P, C], f32, tag="s", name="ps_p")
                        for a in range(2):
                            nc.tensor.matmul(
                                ps_p,
                                lhsT=atT[:, a, t * P:(t + 1) * P],
                                rhs=w_sb["Wp"][:, a, :],
                                start=(a == 0),
                                stop=(a == 1),
                            )
                        f_t = fin_pool.tile([P, C], f32, tag="f")
                        nc.vector.scalar_tensor_tensor(
                            out=f_t,
                            in0=ps_p,
                            scalar=inv_ts[blk * 4 + s],
                            in1=xnf[:, t, :],
                            op0=AluOpType.mult,
                            op1=AluOpType.add,
                        )
                        nc.sync.dma_start(
                            out=out_d[t * P:(t + 1) * P, :], in_=f_t
                        )
                  return t_proj

                allp = [(blk, s) for blk in range(2) for s in range(IB // P)]
                for k in range(4):
                    tasks[10 + k] = mk_proj(allp[2 * k:2 * k + 2])
                return tasks

            po0 = run_superblock(0)
            invs0 = early_epilogue(0, po0)
            po1 = run_superblock(1, tasks=late_tasks(0, invs0))
            invs1 = early_epilogue(1, po1)
            for k in sorted(t1 := late_tasks(1, invs1)):
                t1[k]()

    nc.compile()
    return nc


def _get_nc():
    if "nc" not in _CACHE:
        _CACHE["nc"] = _build_bass()
    return _CACHE["nc"]


def make_in_maps(inputs):
    x = np.ascontiguousarray(np.asarray(inputs["inputs"], np.float32)).reshape(4, NK, C)
    ctx = np.ascontiguousarray(np.asarray(inputs["context"], np.float32)).reshape(4, NK, C)
    f32 = np.float32
    gamma = np.asarray(inputs["gamma"], f32)
    beta = np.asarray(inputs["beta"], f32)
    Wq = np.asarray(inputs["Wq"], f32)
    # fold gamma/beta into the q path: q = xn_raw @ (diag(gamma) Wq) + (beta@Wq + bq)
    Wq_eff = np.ascontiguousarray(gamma[:, None] * Wq)
    bq_eff = np.ascontiguousarray(beta @ Wq + np.asarray(inputs["bq"], f32))
    # residual: xn_raw * gamma + (beta + bp) + proj
    beta_eff = np.ascontiguousarray(beta + np.asarray(inputs["bp"], f32))
    shared = {
        "Wq": Wq_eff,
        "bq": bq_eff,
        "beta": beta_eff,
        "gamma": np.ascontiguousarray(gamma),
        "Wk": np.ascontiguousarray(np.asarray(inputs["Wk"], f32)),
        "bk": np.ascontiguousarray(np.asarray(inputs["bk"], f32)),
        "Wv": np.ascontiguousarray(np.asarray(inputs["Wv"], f32)),
        "bv": np.ascontiguousarray(np.asarray(inputs["bv"], f32)),
        "Wp": np.ascontiguousarray(np.asarray(inputs["Wp"], f32)),
    }
    ctxT_b = [np.ascontiguousarray(ctx[b].T) for b in range(4)]
    in_maps = []
    for core in range(8):
        b, h = divmod(core, 2)
        m = dict(shared)
        m["x"] = np.ascontiguousarray(x[b, h * NQ:(h + 1) * NQ])
        m["ctxT"] = ctxT_b[b]
        in_maps.append(m)
    return in_maps


def kernel(**inputs):
    global LAST_RESULTS
    import os
    if os.environ.get("BASS_TRACE"):
        # run_bass_kernel_spmd's trace path hard-imports antenv.axon_hooks,
        # which not every image ships; shim it so tracing degrades gracefully.
        try:
            import antenv.axon_hooks  # noqa: F401
        except ImportError:
            import sys
            import types

            mod = types.ModuleType("antenv.axon_hooks")
            mod.get_axon_ntff_profile_hook = lambda: None
            mod.set_axon_ntff_profile_hook = lambda h: None
            sys.modules["antenv.axon_hooks"] = mod
    from concourse.bass_utils import run_bass_kernel_spmd

    nc = _get_nc()
    in_maps = make_in_maps(inputs)
    res = run_bass_kernel_spmd(nc, in_maps, core_ids=list(range(8)))
    LAST_RESULTS = res
    full = np.empty((4, NK, C), np.float32)
    for core in range(8):
        b, h = divmod(core, 2)
        full[b, h * NQ:(h + 1) * NQ] = res.results[core]["out"]
    return full.reshape(4, 64, 64, 256)
